# revision 4
# baseline (speedup 1.0000x reference)
"""LogoAwareAttention Trainium2 kernel.

Key observation: the "logo bias" (geo_bias*geometric + txt_bias*text +
col_bias*color) has shape [B, H, 1, 1] -- constant along the softmax axis.
softmax(x + c) == softmax(x) for per-row-constant c, so the bias is a
mathematical no-op and the module is plain multi-head attention:

    y = softmax((x Wq)(x Wk)^T / sqrt(Dh)) (x Wv) Wproj + b_proj

Sharding: data-parallel over batch. B=8 -> one batch element per NeuronCore.

Per-core plan (N=1024 tokens, C=768, H=12 heads, Dh=64), all matmuls bf16
with fp32 PSUM accumulation:
  1. QKV.  xT (c-on-partitions) serves both as the moving operand for
     Q^T/K^T (feature-major) and the stationary operand for V (token-major).
       Q^T,K^T: [feat 128-tile, tok] ; V: [tok 128-tile, feat]
     1/sqrt(Dh) is folded into the Q columns of W_qkv on the host.
  2. Attention per head h: S^T[j,i] = (K_h^T stationary) x (Q_h^T moving),
     j=keys on partitions, i=queries on free.  exp on the scalar engine
     (no max subtraction needed: |scores| <= ~4 here).  P^T (bf16) is the
     moving operand of the PV matmul with stationary [V_h | ones] so PSUM
     rows 0..63 accumulate the unnormalized output^T and row 64 the softmax
     denominator.  recip on DVE, broadcast across partitions via a K=1
     matmul with a ones[1,64] stationary, normalize on DVE -> out^T bf16.
  3. Projection: out^T tiles are exactly the stationary lhsT for the final
     projection; bias comes pre-broadcast [128,768] from the host.
"""

import numpy as np
import ml_dtypes

import concourse.bass as bass
import concourse.tile as tile
from concourse import bacc, mybir
from concourse.bass_utils import run_bass_kernel_spmd

BF16 = mybir.dt.bfloat16
F32 = mybir.dt.float32
NP_BF16 = ml_dtypes.bfloat16

N = 1024          # tokens
C = 768           # channels
H = 12            # heads
DH = 64           # head dim
CT = C // 128     # 6 c-chunks
TT = N // 128     # 8 token tiles
FQK = 2 * C       # q+k feature count (1536)
FT_QK = FQK // 128  # 12 feature tiles for q|k


def _build_nc(taps=False, reps=1):
    nc = bacc.Bacc("TRN2", target_bir_lowering=False, debug=False)

    xt_d = nc.dram_tensor("xt", [C, N], BF16, kind="ExternalInput")
    wqkv_d = nc.dram_tensor("wqkv", [C, 3 * C], BF16, kind="ExternalInput")
    wproj_d = nc.dram_tensor("wproj", [C, C], BF16, kind="ExternalInput")
    bias_d = nc.dram_tensor("bias", [128, C], F32, kind="ExternalInput")
    y_d = nc.dram_tensor("y", [N, C], F32, kind="ExternalOutput")
    if taps:
        tap_qk = nc.dram_tensor("tap_qk", [128, N], BF16, kind="ExternalOutput")
        tap_v = nc.dram_tensor("tap_v", [128, H * 65], BF16, kind="ExternalOutput")
        tap_p = nc.dram_tensor("tap_p", [128, N], BF16, kind="ExternalOutput")
        tap_acc = nc.dram_tensor("tap_acc", [128, N], F32, kind="ExternalOutput")
        tap_out = nc.dram_tensor("tap_out", [128, N], BF16, kind="ExternalOutput")

    with tile.TileContext(nc) as tc:
        with tc.tile_pool(name="const", bufs=1) as cpool, \
             tc.tile_pool(name="qkv", bufs=1) as qkvpool, \
             tc.tile_pool(name="work", bufs=4) as wpool, \
             tc.tile_pool(name="ps", bufs=2, space="PSUM") as pspool, \
             tc.tile_pool(name="psacc", bufs=2, space="PSUM") as accpool:
          for _rep in range(reps):
            R = f"r{_rep}_"

            # ---- load inputs -------------------------------------------------
            xt_sb = []
            for i in range(CT):
                t = cpool.tile([128, N], BF16, tag=f"xt{i}")
                nc.sync.dma_start(t[:, 0:512], xt_d[i * 128:(i + 1) * 128, 0:512])
                nc.sync.dma_start(t[:, 512:N], xt_d[i * 128:(i + 1) * 128, 512:N])
                xt_sb.append(t)
            wqkv_sb = []
            for i in range(CT):
                t = cpool.tile([128, 3 * C], BF16, tag=f"wqkv{i}")
                rows = slice(i * 128, (i + 1) * 128)
                # ft0 (q cols 0:128) and ft6 (k cols 768:896) first so the
                # first attention head's QKV matmuls can start ASAP
                nc.sync.dma_start(t[:, 0:128], wqkv_d[rows, 0:128])
                nc.sync.dma_start(t[:, 768:896], wqkv_d[rows, 768:896])
                wqkv_sb.append(t)
            for i in range(CT):
                t = wqkv_sb[i]
                rows = slice(i * 128, (i + 1) * 128)
                nc.sync.dma_start(t[:, 1536:2304], wqkv_d[rows, 1536:2304])  # V
                nc.sync.dma_start(t[:, 128:768], wqkv_d[rows, 128:768])
                nc.sync.dma_start(t[:, 896:1536], wqkv_d[rows, 896:1536])
            wproj_sb = []
            for i in range(CT):
                t = cpool.tile([128, C], BF16, tag=f"wproj{i}")
                nc.sync.dma_start(t[:], wproj_d[i * 128:(i + 1) * 128, :])
                wproj_sb.append(t)
            bias_sb = cpool.tile([128, C], F32, tag="bias")
            nc.sync.dma_start(bias_sb[:], bias_d[:, :])

            ones_sb = cpool.tile([1, 64], BF16, tag="ones")
            nc.vector.memset(ones_sb[:], 1.0)

            # ---- QKV ---------------------------------------------------------
            # Q^T/K^T: feature-major [feat 128-tile, tokens]
            qkT = [None] * FT_QK  # 0..5 = Q heads (2f,2f+1), 6..11 = K
            for ft in (0, 6):
                ps = pspool.tile([128, N], F32, tag="ps")
                for half in range(2):
                    sl = slice(half * 512, (half + 1) * 512)
                    for cc in range(CT):
                        nc.tensor.matmul(
                            ps[:, sl],
                            lhsT=wqkv_sb[cc][:, ft * 128:(ft + 1) * 128],
                            rhs=xt_sb[cc][:, sl],
                            start=(cc == 0), stop=(cc == CT - 1),
                        )
                t = qkvpool.tile([128, N], BF16, tag=f"qk{ft}", name=f"{R}qk{ft}")
                nc.vector.tensor_copy(out=t[:], in_=ps[:])
                qkT[ft] = t

            # V: token-major [tok 128-tile, 12*65] with a ones column per head
            v_sb = []
            for tt in range(TT):
                ps = pspool.tile([128, N], F32, tag="ps")
                for sl in (slice(0, 512), slice(512, 768)):
                    wsl = slice(2 * C + sl.start, 2 * C + sl.stop)
                    for cc in range(CT):
                        nc.tensor.matmul(
                            ps[:, sl],
                            lhsT=xt_sb[cc][:, tt * 128:(tt + 1) * 128],
                            rhs=wqkv_sb[cc][:, wsl],
                            start=(cc == 0), stop=(cc == CT - 1),
                        )
                t = qkvpool.tile([128, H * 65], BF16, tag=f"v{tt}")
                t3 = t[:].rearrange("p (h w) -> p h w", w=65)
                nc.vector.memset(t3[:, :, 64:65], 1.0)
                nc.vector.tensor_copy(
                    out=t3[:, :, 0:64],
                    in_=ps[:, 0:C].rearrange("p (h w) -> p h w", w=64),
                )
                v_sb.append(t)

            if taps:
                nc.sync.dma_start(tap_qk[:, :], qkT[0][:, :])
                nc.sync.dma_start(tap_v[:, :], v_sb[0][:, :])

            # ---- attention ---------------------------------------------------
            # out^T tiles, 2 heads (2*64 rows) per 128-partition tile
            outT = []
            for i in range(CT):
                outT.append(qkvpool.tile([128, N], BF16, tag=f"outT{i}",
                                         name=f"{R}outT{i}"))

            def normalize(h, acc):
                qrows = slice((h % 2) * 64, (h % 2) * 64 + 64)
                recip = wpool.tile([1, N], F32, tag="recip", name=f"{R}recip{h}")
                nc.vector.reciprocal(recip[:], acc[64:65, :])
                recip_bf = wpool.tile([1, N], BF16, tag="recipbf", name=f"{R}recipbf{h}")
                nc.vector.tensor_copy(out=recip_bf[:], in_=recip[:])
                bc = pspool.tile([128, N], F32, tag="ps", name=f"{R}bc{h}")
                for half in range(2):
                    sl = slice(half * 512, (half + 1) * 512)
                    nc.tensor.matmul(
                        bc[0:64, sl],
                        lhsT=ones_sb[:],
                        rhs=recip_bf[:, sl],
                        start=True, stop=True,
                    )
                bc_sb = wpool.tile([64, N], BF16, tag="bcsb", name=f"{R}bcsb{h}")
                nc.vector.tensor_copy(out=bc_sb[:], in_=bc[0:64, :])
                nc.vector.tensor_tensor(
                    out=outT[h // 2][qrows, :],
                    in0=acc[0:64, :],
                    in1=bc_sb[:],
                    op=mybir.AluOpType.mult,
                )

            pending = None

            # re-add the remaining q/k feature tiles before the head loop
            for ft in (1, 7, 2, 8, 3, 9, 4, 10, 5, 11):
                ps = pspool.tile([128, N], F32, tag="ps", name=f"{R}psqk{ft}")
                for half in range(2):
                    sl = slice(half * 512, (half + 1) * 512)
                    for cc in range(CT):
                        nc.tensor.matmul(
                            ps[:, sl],
                            lhsT=wqkv_sb[cc][:, ft * 128:(ft + 1) * 128],
                            rhs=xt_sb[cc][:, sl],
                            start=(cc == 0), stop=(cc == CT - 1),
                        )
                t = qkvpool.tile([128, N], BF16, tag=f"qk{ft}", name=f"{R}qk{ft}")
                nc.vector.tensor_copy(out=t[:], in_=ps[:])
                qkT[ft] = t

            for h in range(H):
                qt = qkT[h // 2]        # q feature tile
                kt = qkT[6 + h // 2]    # k feature tile
                qrows = slice((h % 2) * 64, (h % 2) * 64 + 64)

                acc = accpool.tile([128, N], F32, tag="acc", name=f"{R}acc{h}")
                pTs = [None] * TT
                for jt in range(TT):
                    # stage the previous head's normalize chain through this
                    # head's jt loop so PE never waits on the DVE recip
                    if pending is not None and jt == 0:
                        ph = pending[0]
                        p_recip = wpool.tile([1, N], F32, tag="recip",
                                             name=f"{R}recip{ph}")
                        nc.vector.reciprocal(p_recip[:], pending[1][64:65, :])
                        p_recipbf = wpool.tile([1, N], BF16, tag="recipbf",
                                               name=f"{R}recipbf{ph}")
                        nc.vector.tensor_copy(out=p_recipbf[:], in_=p_recip[:])
                    if pending is not None and jt == 3:
                        p_bc = pspool.tile([128, N], F32, tag="ps",
                                           name=f"{R}bc{ph}")
                        for half in range(2):
                            sl = slice(half * 512, (half + 1) * 512)
                            nc.tensor.matmul(
                                p_bc[0:64, sl],
                                lhsT=ones_sb[:],
                                rhs=p_recipbf[:, sl],
                                start=True, stop=True,
                            )
                    if pending is not None and jt == 5:
                        p_bcsb = wpool.tile([64, N], BF16, tag="bcsb",
                                            name=f"{R}bcsb{ph}")
                        nc.vector.tensor_copy(out=p_bcsb[:], in_=p_bc[0:64, :])
                        nc.vector.tensor_tensor(
                            out=outT[ph // 2][slice((ph % 2) * 64,
                                                    (ph % 2) * 64 + 64), :],
                            in0=pending[1][0:64, :],
                            in1=p_bcsb[:],
                            op=mybir.AluOpType.mult,
                        )
                        pending = None

                    ps = pspool.tile([128, N], F32, tag="ps", name=f"{R}st{h}_{jt}")
                    for half in range(2):
                        sl = slice(half * 512, (half + 1) * 512)
                        nc.tensor.matmul(
                            ps[:, sl],
                            lhsT=kt[qrows, jt * 128:(jt + 1) * 128],
                            rhs=qt[qrows, sl],
                            start=True, stop=True,
                        )
                    pT = wpool.tile([128, N], BF16, tag="pT", name=f"{R}pT{h}_{jt}")
                    nc.scalar.activation(pT[:], ps[:], mybir.ActivationFunctionType.Exp)
                    if taps and h == 0 and jt == 0:
                        nc.sync.dma_start(tap_p[:, :], pT[:, :])
                    pTs[jt] = pT
                    if jt >= 1:
                        for half in range(2):
                            sl = slice(half * 512, (half + 1) * 512)
                            nc.tensor.matmul(
                                acc[0:65, sl],
                                lhsT=v_sb[jt - 1][:, h * 65:(h + 1) * 65],
                                rhs=pTs[jt - 1][:, sl],
                                start=(jt - 1 == 0), stop=False,
                            )
                        pTs[jt - 1] = None
                for half in range(2):
                    sl = slice(half * 512, (half + 1) * 512)
                    nc.tensor.matmul(
                        acc[0:65, sl],
                        lhsT=v_sb[TT - 1][:, h * 65:(h + 1) * 65],
                        rhs=pTs[TT - 1][:, sl],
                        start=False, stop=True,
                    )
                if h == H - 1:
                    normalize(h, acc)
                else:
                    pending = (h, acc)

            if pending is not None:
                normalize(*pending)
            if taps:
                nc.sync.dma_start(tap_out[:, :], outT[0][:, :])

            # ---- projection --------------------------------------------------
            for it in range(TT):
                ps = accpool.tile([128, N], F32, tag="acc")  # cols 0..767 used
                for sl in (slice(0, 512), slice(512, 768)):
                    for cc in range(CT):
                        nc.tensor.matmul(
                            ps[:, sl],
                            lhsT=outT[cc][:, it * 128:(it + 1) * 128],
                            rhs=wproj_sb[cc][:, sl],
                            start=(cc == 0), stop=(cc == CT - 1),
                        )
                y_sb = wpool.tile([128, C], F32, tag="ysb")
                nc.vector.tensor_tensor(
                    out=y_sb[:], in0=ps[:, 0:C], in1=bias_sb[:],
                    op=mybir.AluOpType.add,
                )
                nc.sync.dma_start(y_d[it * 128:(it + 1) * 128, :], y_sb[:])

    nc.compile()
    return nc


_NC_CACHE = None


def _get_nc():
    global _NC_CACHE
    if _NC_CACHE is None:
        _NC_CACHE = _build_nc()
    return _NC_CACHE


def _prep_in_maps(x, W_qkv, W_proj, b_proj):
    scale = DH ** -0.5
    wqkv = W_qkv.copy()
    wqkv[:, :C] *= scale
    wqkv_bf = wqkv.astype(NP_BF16)
    wproj_bf = W_proj.astype(NP_BF16)
    bias_f = np.ascontiguousarray(np.broadcast_to(b_proj, (128, C))).astype(np.float32)

    in_maps = []
    for b in range(8):
        xt = np.ascontiguousarray(x[b].T).astype(NP_BF16)
        in_maps.append({"xt": xt, "wqkv": wqkv_bf, "wproj": wproj_bf, "bias": bias_f})
    return in_maps


def kernel(x, geometric, text, color, W_qkv, W_proj, b_proj,
           geo_bias, txt_bias, col_bias, _trace=False, **_ignored):
    x = np.asarray(x, dtype=np.float32)
    W_qkv = np.asarray(W_qkv, dtype=np.float32)
    W_proj = np.asarray(W_proj, dtype=np.float32)
    b_proj = np.asarray(b_proj, dtype=np.float32)

    in_maps = _prep_in_maps(x, W_qkv, W_proj, b_proj)

    nc = _get_nc()
    res = run_bass_kernel_spmd(nc, in_maps, core_ids=list(range(8)), trace=_trace)
    y = np.stack([r["y"] for r in res.results]).astype(np.float32)
    if _trace:
        kernel.last_results = res
    return y



# revision 5
# speedup vs baseline: 3.3909x; 3.3909x over previous
"""LogoAwareAttention Trainium2 kernel.

Key observation: the "logo bias" (geo_bias*geometric + txt_bias*text +
col_bias*color) has shape [B, H, 1, 1] -- constant along the softmax axis.
softmax(x + c) == softmax(x) for per-row-constant c, so the bias is a
mathematical no-op and the module is plain multi-head attention:

    y = softmax((x Wq)(x Wk)^T / sqrt(Dh)) (x Wv) Wproj + b_proj

Sharding: data-parallel over batch. B=8 -> one batch element per NeuronCore.

Per-core plan (N=1024 tokens, C=768, H=12 heads, Dh=64), all matmuls bf16
with fp32 PSUM accumulation:
  1. QKV.  xT (c-on-partitions) serves both as the moving operand for
     Q^T/K^T (feature-major) and the stationary operand for V (token-major).
       Q^T,K^T: [feat 128-tile, tok] ; V: [tok 128-tile, feat]
     1/sqrt(Dh) is folded into the Q columns of W_qkv on the host.
  2. Attention per head h: S^T[j,i] = (K_h^T stationary) x (Q_h^T moving),
     j=keys on partitions, i=queries on free.  exp on the scalar engine
     (no max subtraction needed: |scores| <= ~4 here).  P^T (bf16) is the
     moving operand of the PV matmul with stationary [V_h | ones] so PSUM
     rows 0..63 accumulate the unnormalized output^T and row 64 the softmax
     denominator.  recip on DVE, broadcast across partitions via a K=1
     matmul with a ones[1,64] stationary, normalize on DVE -> out^T bf16.
  3. Projection: out^T tiles are exactly the stationary lhsT for the final
     projection; bias comes pre-broadcast [128,768] from the host.
"""

import numpy as np
import ml_dtypes

import concourse.bass as bass
import concourse.tile as tile
from concourse import bacc, mybir
from concourse.bass_utils import run_bass_kernel_spmd

BF16 = mybir.dt.bfloat16
F32 = mybir.dt.float32
NP_BF16 = ml_dtypes.bfloat16

N = 1024          # tokens
C = 768           # channels
H = 12            # heads
DH = 64           # head dim
CT = C // 128     # 6 c-chunks
TT = N // 128     # 8 token tiles
FQK = 2 * C       # q+k feature count (1536)
FT_QK = FQK // 128  # 12 feature tiles for q|k


def _build_nc(taps=False, reps=1):
    nc = bacc.Bacc("TRN2", target_bir_lowering=False, debug=False)

    xt_d = nc.dram_tensor("xt", [C, N], BF16, kind="ExternalInput")
    wqkv_d = nc.dram_tensor("wqkv", [C, 3 * C], BF16, kind="ExternalInput")
    wproj_d = nc.dram_tensor("wproj", [C, C], BF16, kind="ExternalInput")
    bias_d = nc.dram_tensor("bias", [128, C], F32, kind="ExternalInput")
    y_d = nc.dram_tensor("y", [N, C], F32, kind="ExternalOutput")
    if taps:
        tap_qk = nc.dram_tensor("tap_qk", [128, N], BF16, kind="ExternalOutput")
        tap_v = nc.dram_tensor("tap_v", [128, H * 65], BF16, kind="ExternalOutput")
        tap_p = nc.dram_tensor("tap_p", [128, N], BF16, kind="ExternalOutput")
        tap_acc = nc.dram_tensor("tap_acc", [128, N], F32, kind="ExternalOutput")
        tap_out = nc.dram_tensor("tap_out", [128, N], BF16, kind="ExternalOutput")

    with tile.TileContext(nc) as tc:
        with tc.tile_pool(name="const", bufs=1) as cpool, \
             tc.tile_pool(name="qkv", bufs=1) as qkvpool, \
             tc.tile_pool(name="work", bufs=4) as wpool, \
             tc.tile_pool(name="ps", bufs=2, space="PSUM") as pspool, \
             tc.tile_pool(name="psacc", bufs=2, space="PSUM") as accpool:
          def _body(R):
            # ---- load inputs -------------------------------------------------
            xt_sb = []
            for i in range(CT):
                t = cpool.tile([128, N], BF16, tag=f"xt{i}")
                nc.sync.dma_start(t[:, 0:512], xt_d[i * 128:(i + 1) * 128, 0:512])
                nc.sync.dma_start(t[:, 512:N], xt_d[i * 128:(i + 1) * 128, 512:N])
                xt_sb.append(t)
            wqkv_sb = []
            for i in range(CT):
                t = cpool.tile([128, 3 * C], BF16, tag=f"wqkv{i}")
                rows = slice(i * 128, (i + 1) * 128)
                # ft0 (q cols 0:128) and ft6 (k cols 768:896) first so the
                # first attention head's QKV matmuls can start ASAP
                nc.sync.dma_start(t[:, 0:128], wqkv_d[rows, 0:128])
                nc.sync.dma_start(t[:, 768:896], wqkv_d[rows, 768:896])
                wqkv_sb.append(t)
            for i in range(CT):
                t = wqkv_sb[i]
                rows = slice(i * 128, (i + 1) * 128)
                nc.sync.dma_start(t[:, 1536:2304], wqkv_d[rows, 1536:2304])  # V
                nc.sync.dma_start(t[:, 128:768], wqkv_d[rows, 128:768])
                nc.sync.dma_start(t[:, 896:1536], wqkv_d[rows, 896:1536])
            wproj_sb = []
            for i in range(CT):
                t = cpool.tile([128, C], BF16, tag=f"wproj{i}")
                nc.sync.dma_start(t[:], wproj_d[i * 128:(i + 1) * 128, :])
                wproj_sb.append(t)
            bias_sb = cpool.tile([128, C], F32, tag="bias")
            nc.sync.dma_start(bias_sb[:], bias_d[:, :])

            ones_sb = cpool.tile([1, 64], BF16, tag="ones")
            nc.vector.memset(ones_sb[:], 1.0)

            # ---- QKV ---------------------------------------------------------
            # Q^T/K^T: feature-major [feat 128-tile, tokens]
            qkT = [None] * FT_QK  # 0..5 = Q heads (2f,2f+1), 6..11 = K
            for ft in (0, 6):
                ps = pspool.tile([128, N], F32, tag="ps")
                for half in range(2):
                    sl = slice(half * 512, (half + 1) * 512)
                    for cc in range(CT):
                        nc.tensor.matmul(
                            ps[:, sl],
                            lhsT=wqkv_sb[cc][:, ft * 128:(ft + 1) * 128],
                            rhs=xt_sb[cc][:, sl],
                            start=(cc == 0), stop=(cc == CT - 1),
                        )
                t = qkvpool.tile([128, N], BF16, tag=f"qk{ft}", name=f"{R}qk{ft}")
                nc.vector.tensor_copy(out=t[:], in_=ps[:])
                qkT[ft] = t

            # V: token-major [tok 128-tile, 12*65] with a ones column per head
            v_sb = []
            for tt in range(TT):
                ps = pspool.tile([128, N], F32, tag="ps")
                for sl in (slice(0, 512), slice(512, 768)):
                    wsl = slice(2 * C + sl.start, 2 * C + sl.stop)
                    for cc in range(CT):
                        nc.tensor.matmul(
                            ps[:, sl],
                            lhsT=xt_sb[cc][:, tt * 128:(tt + 1) * 128],
                            rhs=wqkv_sb[cc][:, wsl],
                            start=(cc == 0), stop=(cc == CT - 1),
                        )
                t = qkvpool.tile([128, H * 65], BF16, tag=f"v{tt}")
                t3 = t[:].rearrange("p (h w) -> p h w", w=65)
                nc.vector.memset(t3[:, :, 64:65], 1.0)
                nc.vector.tensor_copy(
                    out=t3[:, :, 0:64],
                    in_=ps[:, 0:C].rearrange("p (h w) -> p h w", w=64),
                )
                v_sb.append(t)

            if taps:
                nc.sync.dma_start(tap_qk[:, :], qkT[0][:, :])
                nc.sync.dma_start(tap_v[:, :], v_sb[0][:, :])

            # ---- attention ---------------------------------------------------
            # out^T tiles, 2 heads (2*64 rows) per 128-partition tile
            outT = []
            for i in range(CT):
                outT.append(qkvpool.tile([128, N], BF16, tag=f"outT{i}",
                                         name=f"{R}outT{i}"))

            def normalize(h, acc):
                qrows = slice((h % 2) * 64, (h % 2) * 64 + 64)
                recip = wpool.tile([1, N], F32, tag="recip", name=f"{R}recip{h}")
                nc.vector.reciprocal(recip[:], acc[64:65, :])
                recip_bf = wpool.tile([1, N], BF16, tag="recipbf", name=f"{R}recipbf{h}")
                nc.vector.tensor_copy(out=recip_bf[:], in_=recip[:])
                bc = pspool.tile([128, N], F32, tag="ps", name=f"{R}bc{h}")
                for half in range(2):
                    sl = slice(half * 512, (half + 1) * 512)
                    nc.tensor.matmul(
                        bc[0:64, sl],
                        lhsT=ones_sb[:],
                        rhs=recip_bf[:, sl],
                        start=True, stop=True,
                    )
                bc_sb = wpool.tile([64, N], BF16, tag="bcsb", name=f"{R}bcsb{h}")
                nc.vector.tensor_copy(out=bc_sb[:], in_=bc[0:64, :])
                nc.vector.tensor_tensor(
                    out=outT[h // 2][qrows, :],
                    in0=acc[0:64, :],
                    in1=bc_sb[:],
                    op=mybir.AluOpType.mult,
                )

            pending = None

            # re-add the remaining q/k feature tiles before the head loop
            for ft in (1, 7, 2, 8, 3, 9, 4, 10, 5, 11):
                ps = pspool.tile([128, N], F32, tag="ps", name=f"{R}psqk{ft}")
                for half in range(2):
                    sl = slice(half * 512, (half + 1) * 512)
                    for cc in range(CT):
                        nc.tensor.matmul(
                            ps[:, sl],
                            lhsT=wqkv_sb[cc][:, ft * 128:(ft + 1) * 128],
                            rhs=xt_sb[cc][:, sl],
                            start=(cc == 0), stop=(cc == CT - 1),
                        )
                t = qkvpool.tile([128, N], BF16, tag=f"qk{ft}", name=f"{R}qk{ft}")
                nc.vector.tensor_copy(out=t[:], in_=ps[:])
                qkT[ft] = t

            for h in range(H):
                qt = qkT[h // 2]        # q feature tile
                kt = qkT[6 + h // 2]    # k feature tile
                qrows = slice((h % 2) * 64, (h % 2) * 64 + 64)

                acc = accpool.tile([128, N], F32, tag="acc", name=f"{R}acc{h}")
                pTs = [None] * TT
                for jt in range(TT):
                    # stage the previous head's normalize chain through this
                    # head's jt loop so PE never waits on the DVE recip
                    if pending is not None and jt == 0:
                        ph = pending[0]
                        p_recip = wpool.tile([1, N], F32, tag="recip",
                                             name=f"{R}recip{ph}")
                        nc.vector.reciprocal(p_recip[:], pending[1][64:65, :])
                        p_recipbf = wpool.tile([1, N], BF16, tag="recipbf",
                                               name=f"{R}recipbf{ph}")
                        nc.vector.tensor_copy(out=p_recipbf[:], in_=p_recip[:])
                    if pending is not None and jt == 3:
                        p_bc = pspool.tile([128, N], F32, tag="ps",
                                           name=f"{R}bc{ph}")
                        for half in range(2):
                            sl = slice(half * 512, (half + 1) * 512)
                            nc.tensor.matmul(
                                p_bc[0:64, sl],
                                lhsT=ones_sb[:],
                                rhs=p_recipbf[:, sl],
                                start=True, stop=True,
                            )
                    if pending is not None and jt == 5:
                        p_bcsb = wpool.tile([64, N], BF16, tag="bcsb",
                                            name=f"{R}bcsb{ph}")
                        nc.vector.tensor_copy(out=p_bcsb[:], in_=p_bc[0:64, :])
                        nc.vector.tensor_tensor(
                            out=outT[ph // 2][slice((ph % 2) * 64,
                                                    (ph % 2) * 64 + 64), :],
                            in0=pending[1][0:64, :],
                            in1=p_bcsb[:],
                            op=mybir.AluOpType.mult,
                        )
                        pending = None

                    ps = pspool.tile([128, N], F32, tag="ps", name=f"{R}st{h}_{jt}")
                    for half in range(2):
                        sl = slice(half * 512, (half + 1) * 512)
                        nc.tensor.matmul(
                            ps[:, sl],
                            lhsT=kt[qrows, jt * 128:(jt + 1) * 128],
                            rhs=qt[qrows, sl],
                            start=True, stop=True,
                        )
                    pT = wpool.tile([128, N], BF16, tag="pT", name=f"{R}pT{h}_{jt}")
                    nc.scalar.activation(pT[:], ps[:], mybir.ActivationFunctionType.Exp)
                    if taps and h == 0 and jt == 0:
                        nc.sync.dma_start(tap_p[:, :], pT[:, :])
                    pTs[jt] = pT
                    if jt >= 1:
                        for half in range(2):
                            sl = slice(half * 512, (half + 1) * 512)
                            nc.tensor.matmul(
                                acc[0:65, sl],
                                lhsT=v_sb[jt - 1][:, h * 65:(h + 1) * 65],
                                rhs=pTs[jt - 1][:, sl],
                                start=(jt - 1 == 0), stop=False,
                            )
                        pTs[jt - 1] = None
                for half in range(2):
                    sl = slice(half * 512, (half + 1) * 512)
                    nc.tensor.matmul(
                        acc[0:65, sl],
                        lhsT=v_sb[TT - 1][:, h * 65:(h + 1) * 65],
                        rhs=pTs[TT - 1][:, sl],
                        start=False, stop=True,
                    )
                if h == H - 1:
                    normalize(h, acc)
                else:
                    pending = (h, acc)

            if pending is not None:
                normalize(*pending)
            if taps:
                nc.sync.dma_start(tap_out[:, :], outT[0][:, :])

            # ---- projection --------------------------------------------------
            for it in range(TT):
                ps = accpool.tile([128, N], F32, tag="acc")  # cols 0..767 used
                for sl in (slice(0, 512), slice(512, 768)):
                    for cc in range(CT):
                        nc.tensor.matmul(
                            ps[:, sl],
                            lhsT=outT[cc][:, it * 128:(it + 1) * 128],
                            rhs=wproj_sb[cc][:, sl],
                            start=(cc == 0), stop=(cc == CT - 1),
                        )
                y_sb = wpool.tile([128, C], F32, tag="ysb")
                nc.vector.tensor_tensor(
                    out=y_sb[:], in0=ps[:, 0:C], in1=bias_sb[:],
                    op=mybir.AluOpType.add,
                )
                nc.sync.dma_start(y_d[it * 128:(it + 1) * 128, :], y_sb[:])

          if reps == 1:
              _body("r0_")
          else:
              with tc.For_i(0, reps) as _i:
                  _body("rl_")

    nc.compile()
    return nc


_NC_CACHE = None


def _get_nc():
    global _NC_CACHE
    if _NC_CACHE is None:
        _NC_CACHE = _build_nc()
    return _NC_CACHE


def _prep_in_maps(x, W_qkv, W_proj, b_proj):
    scale = DH ** -0.5
    wqkv = W_qkv.copy()
    wqkv[:, :C] *= scale
    wqkv_bf = wqkv.astype(NP_BF16)
    wproj_bf = W_proj.astype(NP_BF16)
    bias_f = np.ascontiguousarray(np.broadcast_to(b_proj, (128, C))).astype(np.float32)

    in_maps = []
    for b in range(8):
        xt = np.ascontiguousarray(x[b].T).astype(NP_BF16)
        in_maps.append({"xt": xt, "wqkv": wqkv_bf, "wproj": wproj_bf, "bias": bias_f})
    return in_maps


def kernel(x, geometric, text, color, W_qkv, W_proj, b_proj,
           geo_bias, txt_bias, col_bias, _trace=False, **_ignored):
    x = np.asarray(x, dtype=np.float32)
    W_qkv = np.asarray(W_qkv, dtype=np.float32)
    W_proj = np.asarray(W_proj, dtype=np.float32)
    b_proj = np.asarray(b_proj, dtype=np.float32)

    in_maps = _prep_in_maps(x, W_qkv, W_proj, b_proj)

    nc = _get_nc()
    res = run_bass_kernel_spmd(nc, in_maps, core_ids=list(range(8)), trace=_trace)
    y = np.stack([r["y"] for r in res.results]).astype(np.float32)
    if _trace:
        kernel.last_results = res
    return y

